# revision 12
# baseline (speedup 1.0000x reference)
"""ContextLSTM core on 8 Trainium2 NeuronCores (Bass/Tile, SPMD).

Strategy: weight-stationary model parallelism. Each core holds a 1/8
column-shard of every in-loop weight resident in SBUF (~15.4 MiB); all
cores process all 64 batch rows in transposed layout (features on
partitions, batch on free dim). Four AllGathers per recurrent step
exchange x / h / (u|ctx) / z. priors/cpriors do not feed the recurrence
and run in a column-sharded postpass; embeds @ W_po1 runs in a device
prepass; actions @ W_in + b_in and the Gumbel noise (fixed key 42) are
host-precomputed. All in-loop matmuls are fp32 (the straight-through
Gumbel argmax flips categories for logit errors >~1e-5, so bf16/fp32r
are unusable). sigmoid(x) = 0.5*tanh(0.5x)+0.5 and elu(x<0) = 2t/(1-t)
with t = tanh(x/2) keep every activation in the single exp_and_others
ACT table set (no per-step table reloads).

Assumes resets are all-zero (guaranteed by the problem's input spec).
"""

import os
import numpy as np

T, B = 64, 64
ACTD, EMB, DET, CTX, HID = 6, 1536, 2048, 512, 1024
SD, SC = 32, 32
ZF = SD * SC
M = 8  # cores

XS = ZF // M      # 128: x/u/post/z cols per core
GS = DET // M     # 256: cols per gate per core
CS = CTX // M     # 64:  ctx cols per core
KH = DET // 128   # 16 h chunks
KZ = ZF // 128    # 8 z chunks
KC = CTX // 128   # 4 ctx chunks

T_RUN = int(os.environ.get("KERNEL_T", T))
PG = 4            # postpass timesteps per group
PR = PG * B       # postpass rows per group (256)

_cache = {}


def _build():
    import concourse.bacc as bacc
    import concourse.tile as tile
    import concourse.mybir as mybir

    F32 = mybir.dt.float32
    I32 = mybir.dt.int32
    ALU = mybir.AluOpType
    AF = mybir.ActivationFunctionType
    RG = [list(range(M))]

    nc = bacc.Bacc("TRN2", target_bir_lowering=False, debug=False, num_devices=M)

    def inp(name, shape):
        return nc.dram_tensor(name, shape, F32, kind="ExternalInput")

    Win_s = inp("Win_s", [12, 128, XS])
    Wih_s = inp("Wih_s", [KZ, 128, 4 * GS])
    Whh_s = inp("Whh_s", [KH, 128, 4 * GS])
    Wcand_s = inp("Wcand_s", [KH, 128, CS])
    Wgate_s = inp("Wgate_s", [KH + KC, 128, CS])
    Wpo1h_s = inp("Wpo1h_s", [KH, 128, XS])
    Wpo2_s = inp("Wpo2_s", [KZ, 128, XS])
    Wpo1e_s = inp("Wpo1e_s", [12, 128, XS])
    blstm_s = inp("blstm_s", [8, 128])
    bcand_s = inp("bcand_s", [CS, 1])
    bgate_s = inp("bgate_s", [CS, 1])
    bpo1_s = inp("bpo1_s", [1, XS])
    bpo2_s = inp("bpo2_s", [1, XS])
    Wpr1_s = inp("Wpr1_s", [KH, 128, XS])
    Wpr2_s = inp("Wpr2_s", [KZ, 128, XS])
    Wcp1_s = inp("Wcp1_s", [KC, 128, XS])
    Wcp2_s = inp("Wcp2_s", [KZ, 128, XS])
    bpr1_s = inp("bpr1_s", [1, XS])
    bpr2_s = inp("bpr2_s", [1, XS])
    bcp1_s = inp("bcp1_s", [1, XS])
    bcp2_s = inp("bcp2_s", [1, XS])
    pre_actT = inp("pre_actT", [128, T * B])
    embT = inp("embT", [12, 128, T * B])
    gn_k = inp("gn_k", [T, B, XS])
    h0T = inp("h0T", [KH, 128, B])
    z0T = inp("z0T", [KZ, 128, B])
    c0T = inp("c0T", [KC, 128, B])
    cell0_s = inp("cell0_s", [2, 128, B])
    ctx0m_s = inp("ctx0m_s", [CS, B])

    def outp(name, shape):
        return nc.dram_tensor(name, shape, F32, kind="ExternalOutput")

    DBG = os.environ.get("KERNEL_DBG") == "1"
    if DBG:
        g4dbg_o = outp("g4dbg", [128, 8 * B])
        xdbg_o = outp("xdbg", [128, KZ * B])
    deterT_o = outp("deterT", [T, DET, B])
    ctxT_o = outp("ctxT", [T, CTX, B])
    stoch_o = outp("stoch_k", [T, B, XS])
    posts_o = outp("posts_k", [T, B, XS])
    gates_o = outp("gatesT_k", [T, CS, B])
    cellf_o = outp("cellf_k", [2, 128, B])
    priors_o = outp("priorsT_k", [128, T * B])
    cpriors_o = outp("cpriorsT_k", [128, T * B])

    with tile.TileContext(nc) as tc:
        with (
            tc.tile_pool(name="const", bufs=1) as cp,
            tc.tile_pool(name="dram", bufs=2, space="DRAM") as dp,
        ):
            def wload(pool, dram_t, k, cols, tag):
                t_ = pool.tile([128, k * cols], F32, tag=tag)
                nc.sync.dma_start(
                    t_[:].rearrange("p (k m) -> p k m", k=k),
                    dram_t[:].rearrange("k p m -> p k m"),
                )
                return t_

            # ---- constants ----
            blstm = cp.tile([128, 8], F32, tag="blstm")
            nc.sync.dma_start(blstm[:], blstm_s[:].rearrange("j p -> p j"))
            bcand = cp.tile([CS, 1], F32, tag="bcand")
            nc.sync.dma_start(bcand[:], bcand_s[:])
            bgate = cp.tile([CS, 1], F32, tag="bgate")
            nc.sync.dma_start(bgate[:], bgate_s[:])
            b1 = {}
            for nm, dt_ in (("bpo1", bpo1_s), ("bpo2", bpo2_s), ("bpr1", bpr1_s),
                            ("bpr2", bpr2_s), ("bcp1", bcp1_s), ("bcp2", bcp2_s)):
                b1[nm] = cp.tile([1, XS], F32, tag=nm, name=nm)
                nc.sync.dma_start(b1[nm][:], dt_[:])
            ones = cp.tile([1, PR], F32, tag="ones")
            nc.vector.memset(ones[:], 1.0)
            id_i = cp.tile([128, 128], I32, tag="idi")
            nc.gpsimd.iota(id_i[:], [[1, 128]], base=0, channel_multiplier=-1)
            zt_ = cp.tile([128, 1], F32, tag="zt")
            nc.vector.memset(zt_[:], 0.0)
            idf = cp.tile([128, 128], F32, tag="idf")
            nc.vector.tensor_copy(idf[:], id_i[:])
            ident = cp.tile([128, 128], F32, tag="ident")
            nc.vector.tensor_tensor(ident[:], idf[:], zt_[:].broadcast_to([128, 128]), ALU.is_equal)
            revj_i = cp.tile([B, XS], I32, tag="revji")
            nc.gpsimd.iota(revj_i[:].rearrange("p (g j) -> p g j", g=4),
                           [[0, 4], [-1, 32]], base=32, channel_multiplier=0)
            revj = cp.tile([B, XS], F32, tag="revj")
            nc.vector.tensor_copy(revj[:], revj_i[:])

            # ---- loop-scoped pools: weights, states, work ----
            loop_scope = tc.tile_pool(name="wpool", bufs=1)
            wp = loop_scope.__enter__()
            sp_scope = tc.tile_pool(name="state", bufs=2)
            sp = sp_scope.__enter__()
            kp_scope = tc.tile_pool(name="work", bufs=2)
            kp = kp_scope.__enter__()

            win = wload(wp, Win_s, 12, XS, "win")
            wih = wload(wp, Wih_s, KZ, 4 * GS, "wih")
            whh = wload(wp, Whh_s, KH, 4 * GS, "whh")
            wcand = wload(wp, Wcand_s, KH, CS, "wcand")
            wgate = wload(wp, Wgate_s, KH + KC, CS, "wgate")
            wpo1h = wload(wp, Wpo1h_s, KH, XS, "wpo1h")
            wpo2 = wload(wp, Wpo2_s, KZ, XS, "wpo2")
            pre_po = wp.tile([128, T * B], F32, tag="prepo")
            wpo1e = wload(wp, Wpo1e_s, 12, XS, "wpo1e")

            # ---- prepass: pre_po = embT @ Wpo1e + b_po1 ----
            with (
                tc.tile_pool(name="prestage", bufs=2) as pre_sp,
                tc.tile_pool(name="prepsum", bufs=2, space="PSUM") as pre_pp,
            ):
                RN = 64
                for g in range(T * B // RN):
                    st = pre_sp.tile([128, 12 * RN], F32, tag="embst")
                    nc.sync.dma_start(
                        st[:].rearrange("p (k r) -> p k r", k=12),
                        embT[:, :, RN * g : RN * (g + 1)].rearrange("k p r -> p k r"),
                    )
                    ps = pre_pp.tile([128, RN], F32, tag="preps")
                    for k in range(12):
                        nc.tensor.matmul(ps[:], wpo1e[:, XS * k : XS * (k + 1)],
                                         st[:, RN * k : RN * (k + 1)],
                                         start=(k == 0), stop=False)
                    nc.tensor.matmul(ps[:], b1["bpo1"][:], ones[:, :RN], start=False, stop=True)
                    nc.vector.tensor_copy(pre_po[:, RN * g : RN * (g + 1)], ps[:])

            # ---- recurrent loop ----
            def elu(out_t, s_ap, pfx):
                shp = [s_ap.shape[0], s_ap.shape[1]]
                t1 = kp.tile(shp, F32, tag=pfx + "t1")
                nc.scalar.activation(t1[:], s_ap, AF.Tanh, scale=0.5)
                omt = kp.tile(shp, F32, tag=pfx + "om")
                nc.vector.tensor_scalar(omt[:], t1[:], -1.0, 1.0, ALU.mult, ALU.add)
                rec = kp.tile(shp, F32, tag=pfx + "rc")
                nc.vector.reciprocal(rec[:], omt[:])
                nc.vector.scalar_tensor_tensor(out_t[:], t1[:], 2.0, rec[:], ALU.mult, ALU.mult)
                msk = kp.tile(shp, I32, tag=pfx + "mk")
                nc.vector.tensor_scalar(msk[:], s_ap, 0.0, None, ALU.is_gt)
                nc.vector.copy_predicated(out_t[:], msk[:], s_ap)

            with (
                tc.tile_pool(name="psA", bufs=2, space="PSUM") as psA,
                tc.tile_pool(name="psI", bufs=1, space="PSUM") as psI,
                tc.tile_pool(name="psB", bufs=1, space="PSUM") as psB,
                tc.tile_pool(name="psC", bufs=2, space="PSUM") as psC,
                tc.tile_pool(name="psD", bufs=2, space="PSUM") as psD,
            ):
                hT = sp.tile([128, KH * B], F32, tag="hT")
                nc.sync.dma_start(hT[:].rearrange("p (k b) -> p k b", k=KH),
                                  h0T[:].rearrange("k p b -> p k b"))
                zT = sp.tile([128, KZ * B], F32, tag="zT")
                nc.sync.dma_start(zT[:].rearrange("p (k b) -> p k b", k=KZ),
                                  z0T[:].rearrange("k p b -> p k b"))
                ctxT = sp.tile([128, KC * B], F32, tag="ctxT")
                nc.sync.dma_start(ctxT[:].rearrange("p (k b) -> p k b", k=KC),
                                  c0T[:].rearrange("k p b -> p k b"))
                cell = sp.tile([128, 2 * B], F32, tag="cell")
                nc.sync.dma_start(cell[:].rearrange("p (c b) -> p c b", c=2),
                                  cell0_s[:].rearrange("c p b -> p c b"))
                ctx_mine = sp.tile([CS, B], F32, tag="ctxm")
                nc.sync.dma_start(ctx_mine[:], ctx0m_s[:])

                for t in range(T_RUN):
                    # g4 = x@W_ih + h@W_hh in two psum tiles (each group
                    # contiguous: split groups with interleaved matmuls from
                    # other banks produce corrupted accumulation)
                    g4h = psA.tile([128, 8 * B], F32, tag="g4h")
                    for j in range(8):
                        for k in range(KH):
                            nc.tensor.matmul(
                                g4h[:, B * j : B * (j + 1)],
                                whh[:, k * (4 * GS) + 128 * j : k * (4 * GS) + 128 * (j + 1)],
                                hT[:, B * k : B * (k + 1)],
                                start=(k == 0), stop=(k == KH - 1))

                    # x-MLP: z,ctx contractions -> elu -> x slice
                    psx = psB.tile([128, B], F32, tag="psx")
                    for k in range(KZ):
                        nc.tensor.matmul(psx[:], win[:, XS * k : XS * (k + 1)],
                                         zT[:, B * k : B * (k + 1)],
                                         start=(k == 0), stop=False)
                    for k in range(KC):
                        nc.tensor.matmul(psx[:], win[:, XS * (KZ + k) : XS * (KZ + k + 1)],
                                         ctxT[:, B * k : B * (k + 1)],
                                         start=False, stop=(k == KC - 1))
                    pact = kp.tile([128, B], F32, tag="pact")
                    nc.sync.dma_start(pact[:], pre_actT[:, B * t : B * (t + 1)])
                    sx = kp.tile([128, B], F32, tag="sx")
                    nc.vector.tensor_add(sx[:], psx[:], pact[:])
                    x_mine = kp.tile([128, B], F32, tag="xmine")
                    elu(x_mine, sx[:], "xe")

                    agx_i = dp.tile([128, B], F32, tag="agxi")
                    agx_o = dp.tile([M * 128, B], F32, tag="agxo")
                    nc.sync.dma_start(agx_i[:], x_mine[:])
                    nc.gpsimd.collective_compute("AllGather", ALU.bypass, ins=[agx_i.opt()],
                                                 outs=[agx_o.opt()], replica_groups=RG)
                    xT = sp.tile([128, KZ * B], F32, tag="xT")
                    nc.sync.dma_start(xT[:].rearrange("p (k b) -> p k b", k=KZ),
                                      agx_o[:].rearrange("(k p) b -> p k b", p=128))

                    # g4 W_ih part
                    g4i = psI.tile([128, 8 * B], F32, tag="g4i")
                    for j in range(8):
                        for k in range(KZ):
                            nc.tensor.matmul(
                                g4i[:, B * j : B * (j + 1)],
                                wih[:, k * (4 * GS) + 128 * j : k * (4 * GS) + 128 * (j + 1)],
                                xT[:, B * k : B * (k + 1)],
                                start=(k == 0), stop=(k == KZ - 1))
                    g4 = kp.tile([128, 8 * B], F32, tag="g4s")
                    nc.vector.tensor_copy(g4[:], g4h[:])
                    nc.vector.tensor_add(g4[:], g4[:], g4i[:])

                    if DBG and t == 0:
                        g4dbg = kp.tile([128, 8 * B], F32, tag="g4dbg")
                        nc.vector.tensor_copy(g4dbg[:], g4[:])
                        nc.sync.dma_start(g4dbg_o[:], g4dbg[:])
                        nc.sync.dma_start(xdbg_o[:], xT[:])

                    # LSTM gates (blocks: i=0,1 f=2,3 c=4,5 o=6,7)
                    cell_new = sp.tile([128, 2 * B], F32, tag="cell")
                    h_mine = kp.tile([128, 2 * B], F32, tag="hmine")
                    for c in range(2):
                        g4s = lambda jb: g4[:, B * jb : B * (jb + 1)]
                        ti = kp.tile([128, B], F32, tag="ti")
                        nc.scalar.activation(ti[:], g4s(0 + c), AF.Tanh, bias=blstm[:, 0 + c : 1 + c], scale=0.5)
                        tf = kp.tile([128, B], F32, tag="tf")
                        nc.scalar.activation(tf[:], g4s(2 + c), AF.Tanh, bias=blstm[:, 2 + c : 3 + c], scale=0.5)
                        tcg = kp.tile([128, B], F32, tag="tcg")
                        nc.scalar.activation(tcg[:], g4s(4 + c), AF.Tanh, bias=blstm[:, 4 + c : 5 + c], scale=1.0)
                        to = kp.tile([128, B], F32, tag="to")
                        nc.scalar.activation(to[:], g4s(6 + c), AF.Tanh, bias=blstm[:, 6 + c : 7 + c], scale=0.5)
                        A = kp.tile([128, B], F32, tag="ga")
                        nc.vector.scalar_tensor_tensor(A[:], tf[:], 1.0, cell[:, B * c : B * (c + 1)], ALU.add, ALU.mult)
                        Bv = kp.tile([128, B], F32, tag="gb")
                        nc.vector.scalar_tensor_tensor(Bv[:], ti[:], 1.0, tcg[:], ALU.add, ALU.mult)
                        s_ = kp.tile([128, B], F32, tag="gs")
                        nc.vector.tensor_add(s_[:], A[:], Bv[:])
                        nc.vector.tensor_scalar_mul(cell_new[:, B * c : B * (c + 1)], s_[:], 0.5)
                        tcell = kp.tile([128, B], F32, tag="tcell")
                        nc.scalar.activation(tcell[:], cell_new[:, B * c : B * (c + 1)], AF.Tanh, scale=1.0)
                        nc.vector.scalar_tensor_tensor(h_mine[:, B * c : B * (c + 1)], to[:], 1.0, tcell[:], ALU.add, ALU.mult)
                        nc.vector.tensor_scalar_mul(h_mine[:, B * c : B * (c + 1)], h_mine[:, B * c : B * (c + 1)], 0.5)
                    cell = cell_new

                    agh_i = dp.tile([2 * 128, B], F32, tag="aghi")
                    agh_o = dp.tile([DET, B], F32, tag="agho")
                    nc.sync.dma_start(agh_i[:].rearrange("(c p) b -> p c b", p=128),
                                      h_mine[:].rearrange("p (c b) -> p c b", c=2))
                    nc.gpsimd.collective_compute("AllGather", ALU.bypass, ins=[agh_i.opt()],
                                                 outs=[agh_o.opt()], replica_groups=RG)
                    hT = sp.tile([128, KH * B], F32, tag="hT")
                    nc.sync.dma_start(hT[:].rearrange("p (k b) -> p k b", k=KH),
                                      agh_o[:].rearrange("(k p) b -> p k b", p=128))
                    nc.sync.dma_start(deterT_o[t].rearrange("(k p) b -> p k b", p=128),
                                      hT[:].rearrange("p (k b) -> p k b", k=KH))

                    # cand / gate / ctx update
                    pscg = psC.tile([CS, 2 * B], F32, tag="pscg")
                    for k in range(KH):
                        nc.tensor.matmul(pscg[:, :B], wcand[:, CS * k : CS * (k + 1)],
                                         hT[:, B * k : B * (k + 1)], start=(k == 0), stop=(k == KH - 1))
                    for k in range(KH):
                        nc.tensor.matmul(pscg[:, B:], wgate[:, CS * k : CS * (k + 1)],
                                         hT[:, B * k : B * (k + 1)], start=(k == 0), stop=False)
                    for k in range(KC):
                        nc.tensor.matmul(pscg[:, B:], wgate[:, CS * (KH + k) : CS * (KH + k + 1)],
                                         ctxT[:, B * k : B * (k + 1)], start=False, stop=(k == KC - 1))
                    candT = kp.tile([CS, B], F32, tag="candT")
                    nc.scalar.activation(candT[:], pscg[:, :B], AF.Tanh, bias=bcand[:], scale=1.0)
                    tg = kp.tile([CS, B], F32, tag="tg")
                    nc.scalar.activation(tg[:], pscg[:, B:], AF.Tanh, bias=bgate[:], scale=0.5)
                    gate_sig = kp.tile([CS, B], F32, tag="gsig")
                    nc.vector.tensor_scalar(gate_sig[:], tg[:], 0.5, 0.5, ALU.mult, ALU.add)
                    nc.sync.dma_start(gates_o[t], gate_sig[:])
                    d_ = kp.tile([CS, B], F32, tag="cd")
                    nc.vector.tensor_sub(d_[:], candT[:], ctx_mine[:])
                    gd = kp.tile([CS, B], F32, tag="cgd")
                    nc.vector.tensor_mul(gd[:], gate_sig[:], d_[:])
                    ctx_mine_new = sp.tile([CS, B], F32, tag="ctxm")
                    nc.vector.tensor_add(ctx_mine_new[:], gd[:], ctx_mine[:])
                    ctx_mine = ctx_mine_new

                    # posterior head part 1: u = elu(h@W_po1h + pre_po)
                    psu = psD.tile([128, XS], F32, tag="psD", name="psu")[:, :B]
                    for k in range(KH):
                        nc.tensor.matmul(psu[:], wpo1h[:, XS * k : XS * (k + 1)],
                                         hT[:, B * k : B * (k + 1)], start=(k == 0), stop=(k == KH - 1))
                    su = kp.tile([128, B], F32, tag="su")
                    nc.vector.tensor_add(su[:], psu[:], pre_po[:, B * t : B * (t + 1)])
                    u_mine = kp.tile([128, B], F32, tag="umine")
                    elu(u_mine, su[:], "ue")

                    aguc_i = dp.tile([192, B], F32, tag="aguci")
                    aguc_o = dp.tile([M * 192, B], F32, tag="aguco")
                    nc.sync.dma_start(aguc_i[0:128, :], u_mine[:])
                    nc.sync.dma_start(aguc_i[128:192, :], ctx_mine[:])
                    nc.gpsimd.collective_compute("AllGather", ALU.bypass, ins=[aguc_i.opt()],
                                                 outs=[aguc_o.opt()], replica_groups=RG)
                    uT = sp.tile([128, KZ * B], F32, tag="uT")
                    nc.sync.dma_start(
                        uT[:].rearrange("p (k b) -> p k b", k=KZ),
                        aguc_o[:].rearrange("(k r) b -> k r b", r=192)[:, 0:128].rearrange("k p b -> p k b"))
                    ctxT_new = sp.tile([128, KC * B], F32, tag="ctxT")
                    for hh in range(2):
                        nc.sync.dma_start(
                            ctxT_new[64 * hh : 64 * (hh + 1), :].rearrange("i (c b) -> i c b", c=KC),
                            aguc_o[:].rearrange("(c h r) b -> c h r b", c=KC, h=2)[:, hh, 128:192].rearrange("c i b -> i c b"))
                    ctxT = ctxT_new
                    nc.sync.dma_start(ctxT_o[t].rearrange("(k p) b -> p k b", p=128),
                                      ctxT[:].rearrange("p (k b) -> p k b", k=KC))

                    # posterior part 2: post = u @ W_po2 + b_po2 (natural layout)
                    psp = psD.tile([128, XS], F32, tag="psD", name="psp")[:B, :]
                    for k in range(KZ):
                        nc.tensor.matmul(psp[:], uT[:, B * k : B * (k + 1)],
                                         wpo2[:, XS * k : XS * (k + 1)], start=(k == 0), stop=False)
                    nc.tensor.matmul(psp[:], ones[:1, :B], b1["bpo2"][:], start=False, stop=True)
                    post_sb = kp.tile([B, XS], F32, tag="post")
                    nc.vector.tensor_copy(post_sb[:], psp[:])
                    nc.sync.dma_start(posts_o[t], post_sb[:])

                    # gumbel straight-through sample -> one-hot z (natural)
                    gnt = kp.tile([B, XS], F32, tag="gnt")
                    nc.sync.dma_start(gnt[:], gn_k[t])
                    av = kp.tile([B, XS], F32, tag="av")
                    nc.vector.tensor_add(av[:], psp[:], gnt[:])
                    mx = kp.tile([B, 4], F32, tag="mx")
                    nc.vector.tensor_reduce(mx[:], av[:].rearrange("p (g j) -> p g j", g=4),
                                            mybir.AxisListType.X, ALU.max)
                    msk = kp.tile([B, XS], F32, tag="amsk")
                    for g in range(4):
                        nc.vector.tensor_scalar(msk[:, 32 * g : 32 * (g + 1)],
                                                av[:, 32 * g : 32 * (g + 1)],
                                                mx[:, g : g + 1], None, ALU.is_equal)
                    cc = kp.tile([B, XS], F32, tag="acc")
                    nc.vector.tensor_mul(cc[:], msk[:], revj[:])
                    rr = kp.tile([B, 4], F32, tag="arr")
                    nc.vector.tensor_reduce(rr[:], cc[:].rearrange("p (g j) -> p g j", g=4),
                                            mybir.AxisListType.X, ALU.max)
                    hard = kp.tile([B, XS], F32, tag="hard")
                    for g in range(4):
                        nc.vector.tensor_scalar(hard[:, 32 * g : 32 * (g + 1)],
                                                cc[:, 32 * g : 32 * (g + 1)],
                                                rr[:, g : g + 1], None, ALU.is_equal)
                    nc.sync.dma_start(stoch_o[t], hard[:])

                    # transpose z and gather
                    psz = psD.tile([128, XS], F32, tag="psD", name="psz")[:, :B]
                    nc.tensor.transpose(psz[:], hard[:], ident[:B, :B])
                    z_mine = kp.tile([128, B], F32, tag="zmine")
                    nc.vector.tensor_copy(z_mine[:], psz[:])
                    agz_i = dp.tile([128, B], F32, tag="agzi")
                    agz_o = dp.tile([M * 128, B], F32, tag="agzo")
                    nc.sync.dma_start(agz_i[:], z_mine[:])
                    nc.gpsimd.collective_compute("AllGather", ALU.bypass, ins=[agz_i.opt()],
                                                 outs=[agz_o.opt()], replica_groups=RG)
                    zT = sp.tile([128, KZ * B], F32, tag="zT")
                    nc.sync.dma_start(zT[:].rearrange("p (k b) -> p k b", k=KZ),
                                      agz_o[:].rearrange("(k p) b -> p k b", p=128))

                nc.sync.dma_start(cellf_o[:].rearrange("c p b -> p c b"),
                                  cell[:].rearrange("p (c b) -> p c b", c=2))

            # ---- close loop pools, open postpass pools ----
            kp_scope.__exit__(None, None, None)
            sp_scope.__exit__(None, None, None)
            loop_scope.__exit__(None, None, None)
            with (
                tc.tile_pool(name="wpost", bufs=1) as wq,
                tc.tile_pool(name="poststage", bufs=2) as pp_s,
                tc.tile_pool(name="postwork", bufs=2) as kp,
                tc.tile_pool(name="postpsum", bufs=2, space="PSUM") as pp_p,
            ):
                wpr1 = wload(wq, Wpr1_s, KH, XS, "wpr1")
                wpr2 = wload(wq, Wpr2_s, KZ, XS, "wpr2")
                wcp1 = wload(wq, Wcp1_s, KC, XS, "wcp1")
                wcp2 = wload(wq, Wcp2_s, KZ, XS, "wcp2")
                for g in range(T_RUN // PG):
                    dst = pp_s.tile([128, KH * PR], F32, tag="dstage")
                    for tt in range(PG):
                        nc.sync.dma_start(
                            dst[:].rearrange("p (k t b) -> p k t b", k=KH, t=PG)[:, :, tt],
                            deterT_o[PG * g + tt].rearrange("(k p) b -> p k b", p=128))
                    ps1 = pp_p.tile([128, PR], F32, tag="ps1")
                    for k in range(KH):
                        nc.tensor.matmul(ps1[:], wpr1[:, XS * k : XS * (k + 1)],
                                         dst[:, PR * k : PR * (k + 1)], start=(k == 0), stop=False)
                    nc.tensor.matmul(ps1[:], b1["bpr1"][:], ones[:, :PR], start=False, stop=True)
                    upr_m = kp.tile([128, PR], F32, tag="uprm")
                    elu(upr_m, ps1[:], "pe")
                    agp_i = dp.tile([128, PR], F32, tag="agpi")
                    agp_o = dp.tile([M * 128, PR], F32, tag="agpo")
                    nc.sync.dma_start(agp_i[:], upr_m[:])
                    nc.gpsimd.collective_compute("AllGather", ALU.bypass, ins=[agp_i.opt()],
                                                 outs=[agp_o.opt()], replica_groups=RG)
                    uprT = pp_s.tile([128, KZ * PR], F32, tag="uprT")
                    nc.sync.dma_start(uprT[:].rearrange("p (k b) -> p k b", k=KZ),
                                      agp_o[:].rearrange("(k p) b -> p k b", p=128))
                    ps2 = pp_p.tile([128, PR], F32, tag="ps2")
                    for k in range(KZ):
                        nc.tensor.matmul(ps2[:], wpr2[:, XS * k : XS * (k + 1)],
                                         uprT[:, PR * k : PR * (k + 1)], start=(k == 0), stop=False)
                    nc.tensor.matmul(ps2[:], b1["bpr2"][:], ones[:, :PR], start=False, stop=True)
                    pr_m = kp.tile([128, PR], F32, tag="prm")
                    nc.vector.tensor_copy(pr_m[:], ps2[:])
                    nc.sync.dma_start(priors_o[:, PR * g : PR * (g + 1)], pr_m[:])

                    cst = pp_s.tile([128, KC * PR], F32, tag="cstage")
                    for tt in range(PG):
                        nc.sync.dma_start(
                            cst[:].rearrange("p (k t b) -> p k t b", k=KC, t=PG)[:, :, tt],
                            ctxT_o[PG * g + tt].rearrange("(k p) b -> p k b", p=128))
                    ps3 = pp_p.tile([128, PR], F32, tag="ps1")
                    for k in range(KC):
                        nc.tensor.matmul(ps3[:], wcp1[:, XS * k : XS * (k + 1)],
                                         cst[:, PR * k : PR * (k + 1)], start=(k == 0), stop=False)
                    nc.tensor.matmul(ps3[:], b1["bcp1"][:], ones[:, :PR], start=False, stop=True)
                    ucp_m = kp.tile([128, PR], F32, tag="ucpm")
                    elu(ucp_m, ps3[:], "ce")
                    agc_i = dp.tile([128, PR], F32, tag="agci")
                    agc_o = dp.tile([M * 128, PR], F32, tag="agco")
                    nc.sync.dma_start(agc_i[:], ucp_m[:])
                    nc.gpsimd.collective_compute("AllGather", ALU.bypass, ins=[agc_i.opt()],
                                                 outs=[agc_o.opt()], replica_groups=RG)
                    ucpT = pp_s.tile([128, KZ * PR], F32, tag="ucpT")
                    nc.sync.dma_start(ucpT[:].rearrange("p (k b) -> p k b", k=KZ),
                                      agc_o[:].rearrange("(k p) b -> p k b", p=128))
                    ps4 = pp_p.tile([128, PR], F32, tag="ps2")
                    for k in range(KZ):
                        nc.tensor.matmul(ps4[:], wcp2[:, XS * k : XS * (k + 1)],
                                         ucpT[:, PR * k : PR * (k + 1)], start=(k == 0), stop=False)
                    nc.tensor.matmul(ps4[:], b1["bcp2"][:], ones[:, :PR], start=False, stop=True)
                    cp_m = kp.tile([128, PR], F32, tag="cpm")
                    nc.vector.tensor_copy(cp_m[:], ps4[:])
                    nc.sync.dma_start(cpriors_o[:, PR * g : PR * (g + 1)], cp_m[:])

    nc.compile()
    return nc


def _host_inputs(inputs):
    """Build the 8 per-core input maps from the full problem inputs."""
    import jax

    embeds = np.asarray(inputs["embeds"], np.float32)
    actions = np.asarray(inputs["actions"], np.float32)
    h0 = np.asarray(inputs["h0"], np.float32)
    z0 = np.asarray(inputs["z0"], np.float32)
    context0 = np.asarray(inputs["context0"], np.float32)
    cell0 = np.asarray(inputs["cell0"], np.float32)
    W_in = np.asarray(inputs["W_in"], np.float32)
    b_in = np.asarray(inputs["b_in"], np.float32)
    W_ih = np.asarray(inputs["W_ih"], np.float32)
    W_hh = np.asarray(inputs["W_hh"], np.float32)
    b_lstm = np.asarray(inputs["b_lstm"], np.float32)
    W_cand = np.asarray(inputs["W_cand"], np.float32)
    b_cand = np.asarray(inputs["b_cand"], np.float32)
    W_gate = np.asarray(inputs["W_gate"], np.float32)
    b_gate = np.asarray(inputs["b_gate"], np.float32)
    W_pr1 = np.asarray(inputs["W_pr1"], np.float32)
    b_pr1 = np.asarray(inputs["b_pr1"], np.float32)
    W_pr2 = np.asarray(inputs["W_pr2"], np.float32)
    b_pr2 = np.asarray(inputs["b_pr2"], np.float32)
    W_cp1 = np.asarray(inputs["W_cp1"], np.float32)
    b_cp1 = np.asarray(inputs["b_cp1"], np.float32)
    W_cp2 = np.asarray(inputs["W_cp2"], np.float32)
    b_cp2 = np.asarray(inputs["b_cp2"], np.float32)
    W_po1 = np.asarray(inputs["W_po1"], np.float32)
    b_po1 = np.asarray(inputs["b_po1"], np.float32)
    W_po2 = np.asarray(inputs["W_po2"], np.float32)
    b_po2 = np.asarray(inputs["b_po2"], np.float32)

    # gumbel noise, bit-matching the reference (jax CPU, key 42)
    import jax.numpy as jnp
    cpu = jax.devices("cpu")[0]
    with jax.default_device(cpu):
        keys = jax.random.split(jax.random.key(42), T)
        gn = np.stack([
            np.asarray(-jnp.log(-jnp.log(jax.random.uniform(
                keys[t], (B, SD, SC), jnp.float32, minval=1e-7, maxval=1.0 - 1e-7))))
            for t in range(T)
        ]).astype(np.float32)  # [T,B,SD,SC]

    pre_act = actions.reshape(T * B, ACTD) @ W_in[ZF:ZF + ACTD] + b_in  # [TB, HID]
    pre_act = pre_act.astype(np.float32)
    embT = np.ascontiguousarray(embeds.reshape(T * B, EMB).T).reshape(12, 128, T * B)

    in_maps = []
    for k in range(M):
        xs = slice(XS * k, XS * (k + 1))
        cs = slice(CS * k, CS * (k + 1))
        wih_k = np.concatenate([W_ih[:, g * DET + GS * k : g * DET + GS * (k + 1)] for g in range(4)], 1)
        whh_k = np.concatenate([W_hh[:, g * DET + GS * k : g * DET + GS * (k + 1)] for g in range(4)], 1)
        bl = np.stack([b_lstm[g * DET + GS * k : g * DET + GS * (k + 1)] for g in range(4)])  # [4,256]
        bl = bl * np.array([0.5, 0.5, 1.0, 0.5], np.float32)[:, None]
        m = {
            "Win_s": np.ascontiguousarray(
                np.concatenate([W_in[0:ZF, xs], W_in[ZF + ACTD :, xs]], 0)).reshape(12, 128, XS),
            "Wih_s": np.ascontiguousarray(wih_k).reshape(KZ, 128, 4 * GS),
            "Whh_s": np.ascontiguousarray(whh_k).reshape(KH, 128, 4 * GS),
            "Wcand_s": np.ascontiguousarray(W_cand[:, cs]).reshape(KH, 128, CS),
            "Wgate_s": np.ascontiguousarray(W_gate[:, cs]).reshape(KH + KC, 128, CS),
            "Wpo1h_s": np.ascontiguousarray(W_po1[0:DET, xs]).reshape(KH, 128, XS),
            "Wpo1e_s": np.ascontiguousarray(W_po1[DET:, xs]).reshape(12, 128, XS),
            "Wpo2_s": np.ascontiguousarray(W_po2[:, xs]).reshape(KZ, 128, XS),
            "blstm_s": np.ascontiguousarray(bl).reshape(8, 128),
            "bcand_s": np.ascontiguousarray(b_cand[cs]).reshape(CS, 1),
            "bgate_s": np.ascontiguousarray(0.5 * b_gate[cs]).reshape(CS, 1),
            "bpo1_s": np.ascontiguousarray(b_po1[xs]).reshape(1, XS),
            "bpo2_s": np.ascontiguousarray(b_po2[xs]).reshape(1, XS),
            "Wpr1_s": np.ascontiguousarray(W_pr1[:, xs]).reshape(KH, 128, XS),
            "Wpr2_s": np.ascontiguousarray(W_pr2[:, xs]).reshape(KZ, 128, XS),
            "Wcp1_s": np.ascontiguousarray(W_cp1[:, xs]).reshape(KC, 128, XS),
            "Wcp2_s": np.ascontiguousarray(W_cp2[:, xs]).reshape(KZ, 128, XS),
            "bpr1_s": np.ascontiguousarray(b_pr1[xs]).reshape(1, XS),
            "bpr2_s": np.ascontiguousarray(b_pr2[xs]).reshape(1, XS),
            "bcp1_s": np.ascontiguousarray(b_cp1[xs]).reshape(1, XS),
            "bcp2_s": np.ascontiguousarray(b_cp2[xs]).reshape(1, XS),
            "pre_actT": np.ascontiguousarray(pre_act[:, xs].T),
            "embT": embT,
            "gn_k": np.ascontiguousarray(gn[:, :, 4 * k : 4 * (k + 1), :]).reshape(T, B, XS),
            "h0T": np.ascontiguousarray(h0.T).reshape(KH, 128, B),
            "z0T": np.ascontiguousarray(z0.T).reshape(KZ, 128, B),
            "c0T": np.ascontiguousarray(context0.T).reshape(KC, 128, B),
            "cell0_s": np.ascontiguousarray(cell0.T[GS * k : GS * (k + 1)]).reshape(2, 128, B),
            "ctx0m_s": np.ascontiguousarray(context0.T[cs]),
        }
        in_maps.append(m)
    return in_maps


def kernel(**inputs):
    from concourse import bass_utils

    if "nc" not in _cache:
        _cache["nc"] = _build()
    nc = _cache["nc"]
    in_maps = _host_inputs(inputs)
    res = bass_utils.run_bass_kernel_spmd(nc, in_maps, core_ids=list(range(M)))
    r = res.results
    _cache["last_exec_ns"] = res.exec_time_ns

    deter = np.ascontiguousarray(r[0]["deterT"].transpose(0, 2, 1))        # [T,B,DET]
    ctxs = np.ascontiguousarray(r[0]["ctxT"].transpose(0, 2, 1))           # [T,B,CTX]
    stoch = np.concatenate([r[k]["stoch_k"] for k in range(M)], axis=2)
    posts = np.concatenate([r[k]["posts_k"] for k in range(M)], axis=2)
    gates = np.concatenate([r[k]["gatesT_k"] for k in range(M)], axis=1).transpose(0, 2, 1)
    gates = np.ascontiguousarray(gates)
    priorsT = np.concatenate([r[k]["priorsT_k"] for k in range(M)], axis=0)  # [ZF, TB]
    priors = np.ascontiguousarray(priorsT.T).reshape(T, B, ZF)
    cpriorsT = np.concatenate([r[k]["cpriorsT_k"] for k in range(M)], axis=0)
    cpriors = np.ascontiguousarray(cpriorsT.T).reshape(T, B, ZF)
    features = np.concatenate([deter, stoch, ctxs], axis=-1)
    hf = deter[T - 1].copy()
    zf = stoch[T - 1].copy()
    cf = ctxs[T - 1].copy()
    cellT = np.concatenate([r[k]["cellf_k"].reshape(GS, B) for k in range(M)], axis=0)
    cellf = np.ascontiguousarray(cellT.T)
    return (priors, posts, cpriors, deter, stoch, ctxs, gates, features,
            hf, zf, cf, cellf)
